# revision 12
# baseline (speedup 1.0000x reference)
"""TRN2 Bass kernel for nn_ConditionalInformationCouplingModule.

Single-head cross-attention module with 1x1-conv q/k/v (k/v 2x2-maxpooled),
output 1x1 conv + BatchNorm, gated by a cosine-similarity mask against the
GAP of kv_x, residual-added to x.

Sharding: data-parallel over batch B=8 -> one batch per NeuronCore (8 cores).

v2 changes vs baseline (engine rebalance; baseline was DVE-bound ~87us):
  - K/V conv rhs is AP-permuted to (hl wl pos) quadrant order so the 2x2
    maxpool is two contiguous tensor_tensor(max) ops (DVE L1 from PSUM,
    GpSimd L2 in SBUF) instead of a 2048-elem tensor_reduce per chunk.
  - mask path: 1/sqrt(ng2*s2) = Exp(-0.5*Ln(ng2*s2)); the Ln fuses the s2
    PSUM evac + ng2 scale. Ln/Exp/Identity/Copy/Square share one ACT table
    -> zero table reloads (sqrt never shares a set with exp).
  - gs via DVE tensor_scalar(+accum) 2x mode; squares + x-add on GpSimd.
  - masks computed per-chunk (no sqrt batching needed anymore).
  - W-conv lags attention by 2 chunks so PE never waits on DVE-made y.
  - PSUM: ps [128,1024]x2 for S/convs/W/misc; pzv [128,2,512]x2 for Z|PV.
"""
import sys
import numpy as np

for _p in ('/opt/trn_rl_repo', '/root/.axon_site/_ro/trn_rl_repo'):
    if _p not in sys.path:
        sys.path.append(_p)

B, C, CI, H, W = 8, 256, 128, 64, 64
N = H * W                 # 4096 query positions
M = (H // 2) * (W // 2)   # 1024 key positions
NCH = 512                 # q/n chunk size
NQC = N // NCH            # 8 chunks
NMT = M // 128            # 8 m-tiles
BN_EPS = 1e-5
COS_EPS = 1e-8
EXP_SHIFT = -24.0

_CACHE = {}


def _build(iters=None):
    from contextlib import ExitStack
    import concourse.bacc as bacc
    import concourse.tile as tile
    import concourse.mybir as mybir

    f32 = mybir.dt.float32
    f32r = mybir.dt.float32r
    AF = mybir.ActivationFunctionType
    ALU = mybir.AluOpType
    AX = mybir.AxisListType

    nc = bacc.Bacc("TRN2", target_bir_lowering=False, debug=False,
                   enable_asserts=False, num_devices=B)

    x_d = nc.dram_tensor("x", [2, 128, N], f32r, kind="ExternalInput").ap()
    kv_d = nc.dram_tensor("kv", [2, 128, N], f32r, kind="ExternalInput").ap()
    wq_d = nc.dram_tensor("wq", [128, 256], f32r, kind="ExternalInput").ap()
    wk_d = nc.dram_tensor("wk", [128, 256], f32r, kind="ExternalInput").ap()
    wv_d = nc.dram_tensor("wv", [128, 256], f32r, kind="ExternalInput").ap()
    ww_d = nc.dram_tensor("ww", [128, 256], f32r, kind="ExternalInput").ap()
    qb_d = nc.dram_tensor("qb", [128, 1], f32, kind="ExternalInput").ap()
    kb_d = nc.dram_tensor("kb", [128, 1], f32, kind="ExternalInput").ap()
    vb_d = nc.dram_tensor("vb", [128, 1], f32, kind="ExternalInput").ap()
    wb2_d = nc.dram_tensor("wb2", [128, 2], f32, kind="ExternalInput").ap()
    eye_d = nc.dram_tensor("eye", [128, 128], f32, kind="ExternalInput").ap()
    ones_d = nc.dram_tensor("ones", [128, 128], f32r, kind="ExternalInput").ap()
    out_d = nc.dram_tensor("out", [2, 128, N], f32, kind="ExternalOutput").ap()

    MM = nc.tensor.matmul

    with ExitStack() as ctx:
        tc = ctx.enter_context(tile.TileContext(nc))
        const = ctx.enter_context(tc.tile_pool(name="const", bufs=1))
        qp = ctx.enter_context(tc.tile_pool(name="qp", bufs=4))
        yp = ctx.enter_context(tc.tile_pool(name="yp", bufs=3))
        mp = ctx.enter_context(tc.tile_pool(name="mp", bufs=8))
        sqp = ctx.enter_context(tc.tile_pool(name="sqp", bufs=2))
        l1p = ctx.enter_context(tc.tile_pool(name="l1p", bufs=2))
        up = ctx.enter_context(tc.tile_pool(name="up", bufs=3))
        expp = ctx.enter_context(tc.tile_pool(name="expp", bufs=8))
        zrp = ctx.enter_context(tc.tile_pool(name="zrp", bufs=2))
        outp = ctx.enter_context(tc.tile_pool(name="outp", bufs=4))
        ps = ctx.enter_context(tc.tile_pool(name="ps", bufs=2, space="PSUM"))
        pzv = ctx.enter_context(tc.tile_pool(name="pzv", bufs=2, space="PSUM"))

        # ---- persistent tiles ----
        X = const.tile([128, 2, N], f32r)
        KV = const.tile([128, 2, N], f32r)
        K = const.tile([128, M], f32r)
        V = const.tile([128, M], f32)
        VT = const.tile([128, NMT, 128], f32r)
        gb = const.tile([128, 2, 128], f32r)
        gs = const.tile([128, 2], f32)
        gsc = const.tile([128, N], f32)      # throwaway out for gs accum
        wq = const.tile([128, 256], f32r)
        wk = const.tile([128, 256], f32r)
        wv = const.tile([128, 256], f32r)
        ww = const.tile([128, 256], f32r)
        qb = const.tile([128, 1], f32)
        kb = const.tile([128, 1], f32)
        vb = const.tile([128, 1], f32)
        wb2 = const.tile([128, 2], f32)
        eye = const.tile([128, 128], f32)
        ones = const.tile([128, 128], f32r)
        ng2 = const.tile([128, 1], f32)
        eshift = const.tile([128, 1], f32)
        nc.vector.memset(eshift[:], EXP_SHIFT)

        # ---- weight/bias loads (outside any timing loop) ----
        for t, d in ((wq, wq_d), (wk, wk_d), (wv, wv_d), (ww, ww_d),
                     (qb, qb_d), (kb, kb_d), (vb, vb_d), (wb2, wb2_d),
                     (eye, eye_d), (ones, ones_d)):
            nc.sync.dma_start(t[:], d[:])

        def body():
            qt = [None] * NQC     # per-chunk Q tiles
            yt = [None] * NQC     # per-chunk Y tiles
            mt_ = [None] * NQC    # per-chunk mask tiles
            ext = [None] * NQC    # per-chunk exp tile lists
            zpv_psum = [None] * NQC

            # ---- kv loads first (k/v convs gate attention) ----
            for h in range(4):
                for j in range(2):
                    sl = slice(h * 1024, (h + 1) * 1024)
                    nc.sync.dma_start(KV[:, j, sl], kv_d[j][:, sl])
            for h in range(4):
                for j in range(2):
                    sl = slice(h * 1024, (h + 1) * 1024)
                    nc.sync.dma_start(X[:, j, sl], x_d[j][:, sl])

            # K and V convs + fused 2x2 maxpool (two tensor_tensor maxes:
            # L1 pairs off wl on DVE evacuating PSUM, L2 pairs off hl on
            # GpSimd in SBUF; replaces the 2048-elem tensor_reduce)
            for i in range(NQC):
                ns = slice(i * NCH, (i + 1) * NCH)
                for wt, dst in ((wk, K), (wv, V)):
                    pk = ps.tile([128, NCH], f32, tag="ps")
                    MM(pk[:], wt[:, 0:128], KV[:, 0, ns], start=True, stop=False)
                    MM(pk[:], wt[:, 128:256], KV[:, 1, ns], start=False, stop=True)
                    # psum col order (hp4 hl2 wp32 wl2): L1 reduces off wl
                    # (single PSUM input; PSUM forbids two tensor inputs)
                    a = l1p.tile([128, 256], f32, tag="l1")
                    nc.vector.tensor_reduce(
                        a[:], pk.rearrange("p (a wl) -> p a wl", a=256, wl=2),
                        axis=AX.X, op=ALU.max)
                    # a col order (hp4 hl2 wp32): L2 pairs off hl
                    # (gpsimd has no max opcode -> DVE)
                    a4 = a.rearrange("p (hp hl wp) -> p hp hl wp",
                                     hp=4, hl=2, wp=32)
                    kd = dst[:, i * 128:(i + 1) * 128].rearrange(
                        "p (hp o wp) -> p hp o wp", hp=4, o=1, wp=32)
                    nc.vector.tensor_tensor(kd, a4[:, :, 0:1, :],
                                            a4[:, :, 1:2, :], ALU.max)
            # + bias (after pool: maxpool(a)+b == maxpool(a+b))
            nc.scalar.activation(K[:], K[:], AF.Identity, bias=kb[:, 0:1])
            nc.scalar.activation(V[:], V[:], AF.Identity, bias=vb[:, 0:1])

            # V^T via PE transposes into one PSUM tile, single ACT evac
            vtp = ps.tile([128, 2 * NCH], f32, tag="ps")
            for t in range(NMT):
                nc.tensor.transpose(vtp[:, t * 128:(t + 1) * 128],
                                    V[:, t * 128:(t + 1) * 128], eye[:])
            nc.scalar.activation(VT[:].rearrange("p t b -> p (t b)"),
                                 vtp[:], AF.Identity)

            # gap (channel means of kv) -> broadcast lhsT + ng2
            for j in range(2):
                nc.vector.tensor_scalar(gsc[:], KV[:, j, :], 1.0, None,
                                        ALU.mult, ALU.add,
                                        accum_out=gs[:, j:j + 1])
            for j in range(2):
                nc.vector.tensor_scalar(gb[:, j, :], ones[:], gs[:, j:j + 1],
                                        1.0 / float(N), ALU.mult, ALU.mult)
            png = ps.tile([128, 8], f32, tag="ps")
            MM(png[:], gb[:, 0, :], gb[:, 0, 0:8], start=True, stop=False)
            MM(png[:], gb[:, 1, :], gb[:, 1, 0:8], start=False, stop=True)
            nc.scalar.activation(ng2[:], png[:, 0:1], AF.Copy)

            def mask_path(i):
                # mask = (gap.x) * exp(-0.5*ln(ng2 * sum_c x^2)); all ACT
                # funcs used are in the natural_log_exp table -> no reloads
                ns = slice(i * NCH, (i + 1) * NCH)
                sq = sqp.tile([128, 2, NCH], f32r, tag="sq")
                for j in range(2):
                    nc.gpsimd.tensor_mul(sq[:, j, :], X[:, j, ns], X[:, j, ns])
                pm = ps.tile([128, 2, NCH], f32, tag="ps")
                MM(pm[:, 0, :], ones[:], sq[:, 0, :], start=True, stop=False)
                MM(pm[:, 0, :], ones[:], sq[:, 1, :], start=False, stop=True)
                MM(pm[:, 1, :], gb[:, 0, :], X[:, 0, ns], start=True, stop=False)
                MM(pm[:, 1, :], gb[:, 1, :], X[:, 1, ns], start=False, stop=True)
                lnd = up.tile([128, NCH], f32, tag="u")
                nc.scalar.activation(lnd[:], pm[:, 0, :], AF.Ln,
                                     scale=ng2[:, 0:1])
                u4 = up.tile([128, NCH], f32, tag="u")
                nc.scalar.activation(u4[:], lnd[:], AF.Exp, scale=-0.5)
                m = mp.tile([128, NCH], f32r, tag="mp")
                nc.vector.tensor_tensor(m[:], pm[:, 1, :], u4[:], ALU.mult)
                mt_[i] = m

            def qconv(i):
                ns = slice(i * NCH, (i + 1) * NCH)
                pq = ps.tile([128, NCH], f32, tag="ps")
                MM(pq[:], wq[:, 0:128], X[:, 0, ns], start=True, stop=False)
                MM(pq[:], wq[:, 128:256], X[:, 1, ns], start=False, stop=True)
                q = qp.tile([128, NCH], f32r, tag="qp")
                nc.scalar.activation(q[:], pq[:], AF.Identity, bias=qb[:, 0:1])
                qt[i] = q

            def zpv_mms(qc, mts):
                # Z and PV accumulation matmuls for chunk qc, m-tiles mts
                zpv = zpv_psum[qc]
                for mt in mts:
                    exs = ext[qc][mt // 2][:, (mt % 2) * NCH:(mt % 2 + 1) * NCH]
                    MM(zpv[:, 0, :], ones[:], exs,
                       start=(mt == 0), stop=(mt == NMT - 1))
                    MM(zpv[:, 1, :], VT[:, mt, :], exs,
                       start=(mt == 0), stop=(mt == NMT - 1))

            def zpv_fin(qc):
                zpv = zpv_psum[qc]
                zr = zrp.tile([128, NCH], f32, tag="zr")
                nc.vector.reciprocal(zr[:], zpv[:, 0, :])
                y = yp.tile([128, NCH], f32r, tag="yp")
                nc.vector.tensor_tensor(y[:], zpv[:, 1, :], zr[:], ALU.mult)
                yt[qc] = y

            def attn_iter(qc):
                # S+exp for chunk qc; Z/PV matmuls for chunk qc-1 interleaved
                # into the exp-wait bubbles of the S stream.
                zpv = pzv.tile([128, 2, NCH], f32, tag="zpv")
                zpv_psum[qc] = zpv
                exs = []
                for half in range(NMT // 2):
                    sp = ps.tile([128, 2 * NCH], f32, tag="ps")
                    for sub in range(2):
                        mt = half * 2 + sub
                        MM(sp[:, sub * NCH:(sub + 1) * NCH],
                           K[:, mt * 128:(mt + 1) * 128], qt[qc][:],
                           start=True, stop=True)
                    ex = expp.tile([128, 2 * NCH], f32r, tag="ex")
                    nc.scalar.activation(ex[:], sp[:], AF.Exp, bias=eshift[:, 0:1])
                    exs.append(ex)
                    if qc > 0:
                        zpv_mms(qc - 1, [half * 2, half * 2 + 1])
                ext[qc] = exs
                if qc > 0:
                    zpv_fin(qc - 1)

            def wchunk(qc):
                qs = slice(qc * NCH, (qc + 1) * NCH)
                pw = ps.tile([128, 2, NCH], f32, tag="ps")
                for cc in range(2):
                    MM(pw[:, cc, :], ww[:, cc * 128:(cc + 1) * 128], yt[qc][:],
                       start=True, stop=True)
                for cc in range(2):
                    ot = outp.tile([128, NCH], f32, tag="ot")
                    nc.vector.scalar_tensor_tensor(ot[:], pw[:, cc, :],
                                                   wb2[:, cc:cc + 1],
                                                   mt_[qc][:], ALU.add, ALU.mult)
                    nc.gpsimd.tensor_add(ot[:], ot[:], X[:, cc, qs])
                    nc.sync.dma_start(out_d[cc][:, qs], ot[:])

            for i in range(4):
                mask_path(i)
            qconv(0)
            qconv(1)
            # ---- software-pipelined main loop over query chunks ----
            for qc in range(NQC):
                if qc >= 2:
                    wchunk(qc - 2)
                attn_iter(qc)
                if qc + 2 < NQC:
                    qconv(qc + 2)
                if qc < 4:
                    mask_path(qc + 4)
            zpv_mms(NQC - 1, list(range(NMT)))
            zpv_fin(NQC - 1)
            wchunk(NQC - 2)
            wchunk(NQC - 1)

        if iters is None:
            body()
        else:
            with tc.For_i(0, iters, 1, hint_engines=(mybir.EngineType.PE,)):
                body()

    nc.compile()
    return nc


def _prep_shared(inputs):
    f = np.float32
    Wq, Wk, Wv, Ww = (np.asarray(inputs[k], f) for k in ("Wq", "Wk", "Wv", "Ww"))
    bq, bk, bv, bw = (np.asarray(inputs[k], f) for k in ("bq", "bk", "bv", "bw"))
    gamma, beta = np.asarray(inputs["bn_gamma"], f), np.asarray(inputs["bn_beta"], f)
    mean, var = np.asarray(inputs["bn_mean"], f), np.asarray(inputs["bn_var"], f)

    def pack_T(w):  # [Ci=128, C=256] -> lhsT chunks packed [128, 256]
        wT = np.ascontiguousarray(w.T)          # [256, 128]
        return np.concatenate([wT[:128], wT[128:]], axis=1)  # [128, 256]

    inv = gamma / np.sqrt(var + np.float32(BN_EPS))
    ww_fold = (inv[:, None] * Ww)               # [256, 128]
    shared = {
        "wq": pack_T(Wq), "wk": pack_T(Wk), "wv": pack_T(Wv),
        "ww": np.ascontiguousarray(ww_fold.T),  # [128, 256]
        "qb": bq.reshape(128, 1).copy(), "kb": bk.reshape(128, 1).copy(),
        "vb": bv.reshape(128, 1).copy(),
        "wb2": np.ascontiguousarray((inv * (bw - mean) + beta).reshape(2, 128).T),
        "eye": np.eye(128, dtype=f),
        "ones": np.ones((128, 128), dtype=f),
    }
    return {k: np.ascontiguousarray(v, f) for k, v in shared.items()}


def _make_in_maps(inputs):
    x = np.asarray(inputs["x"], np.float32)
    kv_x = np.asarray(inputs["kv_x"], np.float32)
    shared = _prep_shared(inputs)
    in_maps = []
    for b in range(B):
        m = dict(shared)
        m["x"] = np.ascontiguousarray(x[b].reshape(2, 128, N))
        m["kv"] = np.ascontiguousarray(kv_x[b].reshape(2, 128, N))
        in_maps.append(m)
    return in_maps


def kernel(**inputs):
    import concourse.bass_utils as bass_utils

    if "nc" not in _CACHE:
        _CACHE["nc"] = _build()
    nc = _CACHE["nc"]

    in_maps = _make_in_maps(inputs)
    res = bass_utils.run_bass_kernel_spmd(nc, in_maps, core_ids=list(range(B)))
    out = np.stack([r["out"].reshape(C, H, W) for r in res.results])
    return out.astype(np.float32)
